# revision 14
# baseline (speedup 1.0000x reference)
"""Fused multi-LoRA linear layer on 8 TRN2 NeuronCores.

out = x @ W.T + b + scale * mask(x @ A_all^T) @ B_flat

Sharding: tokens are grouped by adapter on the host. Core i receives 3584
tokens of one assigned adapter (7 "pure" chunks) plus 512 leftover tokens of
mixed adapters (1 "mixed" chunk). The LoRA update for the assigned adapter is
merged into the weight on the host (W'_a = W + scale*B_a@A_a), so pure chunks
are a plain GEMM; only the mixed chunk runs the dense down-projection +
mask-select + up-projection, with mask' = scale*(onehot(sel) - onehot(assigned))
correcting the merged weight to each token's true adapter.

Precision split: the first KF=512 contraction dims run in fp8-e4m3 DoubleRow
(2 k-tiles per matmul, 2x PE throughput), accumulated in a second PSUM bank
and dequantized during eviction:

  t1 = p8 * 2^-8 + bias     (Scalar engine; x was pre-scaled by 2, W' by 128)
  ot = t1 + po              (Vector engine; po = bf16-part PSUM)

Numerically validated: total rel err ~1.6e-2 vs the 2e-2 gate (bf16-only
reference point is ~2e-3).

Device-side layout: the kernel computes out^T [d_out, tokens] so the bias and
fp8 dequant are per-partition scalars and nothing needs an on-chip transpose.
The host applies the inverse token permutation when gathering the output.
"""

import numpy as np
import ml_dtypes

# Problem constants (hardcoded per harness contract).
N, D_IN, D_OUT, L, R = 32768, 2048, 2048, 8, 16
SCALE = 32.0 / 16.0
M_CORES = 8
NS = N // M_CORES  # 4096 tokens per core
P = 128
KT = D_IN // P  # 16 k-tiles total
KF = 512  # contraction dims computed in fp8 DoubleRow
K8T = KF // P  # 4 fp8 k-tiles
KBT = KT - K8T  # 12 bf16 k-tiles
OI = D_OUT // P  # 16 output row-chunks of 128
TW = 512  # token tile width (moving free dim)
TC = NS // TW  # 8 token chunks per core
LR = L * R  # 128
WG = 4  # W column groups
WGC = D_OUT // WG  # 512 columns per group
MIX_T = 1  # chunk slot holding the mixed-adapter leftover tokens
PURE = (TC - 1) * TW  # 3584 single-adapter tokens per core

XSCALE = 2.0  # x pre-scale before e4m3 quantization
WSCALE = 128.0  # W' pre-scale before e4m3 quantization
DEQ = 1.0 / (XSCALE * WSCALE)

_BF16 = ml_dtypes.bfloat16
_F8 = ml_dtypes.float8_e4m3fn

_CACHE = {}

LAST_EXEC_TIME_NS = None


def _build(correct_all):
    import concourse.bass as bass  # noqa: F401
    import concourse.tile as tile
    from concourse import bacc, mybir
    from contextlib import ExitStack

    bf16 = mybir.dt.bfloat16
    f32 = mybir.dt.float32
    f8 = mybir.dt.float8e4
    use_fp8 = not correct_all

    nc = bacc.Bacc(
        "TRN2",
        target_bir_lowering=False,
        debug=False,
        num_devices=M_CORES,
    )

    # Host-prepared, partition-major layouts (see kernel()). In the fp8
    # variant the x/W tensors are split at k=KF into an fp8 part and a bf16
    # part; the fallback (correct_all) variant is all-bf16 with correction
    # masks on every chunk.
    kbt = KBT if use_fp8 else KT
    xT = nc.dram_tensor("xT", [TC, P, kbt, TW], bf16, kind="ExternalInput").ap()
    wT = nc.dram_tensor("wT", [WG, P, kbt, WGC], bf16, kind="ExternalInput").ap()
    if use_fp8:
        x8T = nc.dram_tensor("x8T", [TC, P, K8T, TW], f8, kind="ExternalInput").ap()
        w8T = nc.dram_tensor("w8T", [P, K8T, D_OUT], f8, kind="ExternalInput").ap()
        xM = nc.dram_tensor("xM", [P, K8T, TW], bf16, kind="ExternalInput").ap()
    aT = nc.dram_tensor("aT", [P, KT, LR], bf16, kind="ExternalInput").ap()
    bF = nc.dram_tensor("bF", [P, D_OUT], bf16, kind="ExternalInput").ap()
    bias = nc.dram_tensor("bias", [P, OI], f32, kind="ExternalInput").ap()
    mshape = [TC, P, TW] if correct_all else [P, TW]
    mT = nc.dram_tensor("mT", mshape, bf16, kind="ExternalInput").ap()
    outT = nc.dram_tensor("outT", [D_OUT, NS], f32, kind="ExternalOutput").ap()

    from concourse.tile_rust import add_dep_helper

    with tile.TileContext(nc) as tc, ExitStack() as ctx:
        warm_pool = ctx.enter_context(tc.tile_pool(name="warm", bufs=1))
        wt_pool = ctx.enter_context(tc.tile_pool(name="wt", bufs=WG))
        at_pool = ctx.enter_context(tc.tile_pool(name="at", bufs=1))
        bf_pool = ctx.enter_context(tc.tile_pool(name="bfp", bufs=1))
        bias_pool = ctx.enter_context(tc.tile_pool(name="bias", bufs=1))
        mask_pool = ctx.enter_context(tc.tile_pool(name="mask", bufs=1))
        x_pool = ctx.enter_context(tc.tile_pool(name="x", bufs=2))
        u_pool = ctx.enter_context(tc.tile_pool(name="u", bufs=2))
        o_bufs = 16 if use_fp8 else 4
        o_pool = ctx.enter_context(tc.tile_pool(name="o", bufs=o_bufs))
        if use_fp8:
            w8_pool = ctx.enter_context(tc.tile_pool(name="w8", bufs=1))
            x8_pool = ctx.enter_context(tc.tile_pool(name="x8", bufs=2))
            xm_pool = ctx.enter_context(tc.tile_pool(name="xm", bufs=1))
        pu_bufs = 2 if correct_all else 1
        pu_pool = ctx.enter_context(tc.tile_pool(name="pu", bufs=pu_bufs, space="PSUM"))
        po_pool = ctx.enter_context(tc.tile_pool(name="po", bufs=4, space="PSUM"))
        if use_fp8:
            p8_pool = ctx.enter_context(tc.tile_pool(name="p8", bufs=3, space="PSUM"))

        # Warm up the PE (HAM clock ramp) with throwaway matmuls while the
        # input DMAs stream in; sized to bridge until the first x chunk and
        # W group land (~19us). Rotating accumulation chains over po banks
        # keep the warm matmuls back-to-back.
        warm = warm_pool.tile([P, P], bf16)
        nc.vector.memset(warm[:], 0.0)
        NWB = 3
        pws = [
            po_pool.tile([P, TW], mybir.dt.float32, name="po") for i in range(NWB)
        ]
        NWARM = 112 if use_fp8 else 180
        for i in range(NWARM):
            nc.tensor.matmul(
                pws[i % NWB][:, :P],
                warm[:],
                warm[:],
                start=(i < NWB),
                stop=(i >= NWARM - NWB),
            )

        # Sync ring: first x chunk heads the queue (fp8 slice first - it is
        # small and unblocks the DoubleRow matmuls early).
        if use_fp8:
            x8c0 = x8_pool.tile([P, K8T, TW], f8)
            nc.sync.dma_start(x8c0[:], x8T[0])
        bias_t = bias_pool.tile([P, OI], f32)
        nc.sync.dma_start(bias_t[:], bias[:, :])
        xc0 = x_pool.tile([P, kbt, TW], bf16)
        xc0_dma = nc.sync.dma_start(xc0[:], xT[0])
        at = at_pool.tile([P, KT, LR], bf16)
        nc.sync.dma_start(at[:], aT[:, :, :])
        if use_fp8:
            xm_t = xm_pool.tile([P, K8T, TW], bf16)
            nc.sync.dma_start(xm_t[:], xM[:, :, :])
        if correct_all:
            mask_t = mask_pool.tile([P, TC, TW], bf16)
            nc.sync.dma_start(mask_t[:], mT.rearrange("t p j -> p t j"))
        else:
            mask_t = mask_pool.tile([P, TW], bf16)
            nc.sync.dma_start(mask_t[:], mT[:, :])

        # Scalar ring: fp8 W first (small - it feeds the DoubleRow matmuls
        # that bridge warmup to the bf16 stream), then W' group 0 + B_flat.
        # Remaining W' groups go via the GpSimd SWDGE ring so their issue
        # backpressure never blocks the Scalar engine's evictions; they are
        # gated behind the first x chunk so they don't starve it on HBM.
        wts = []
        for g in range(WG):
            wt_g = wt_pool.tile([P, kbt, WGC], bf16)
            if g == 0:
                if use_fp8:
                    w8 = w8_pool.tile([P, K8T, D_OUT], f8)
                    nc.scalar.dma_start(w8[:], w8T[:, :, :])
                nc.scalar.dma_start(wt_g[:], wT[g])
                bf_t = bf_pool.tile([P, D_OUT], bf16)
                nc.scalar.dma_start(bf_t[:], bF[:, :])
            else:
                wg_dma = nc.gpsimd.dma_start(wt_g[:], wT[g])
                if g == 1:
                    add_dep_helper(
                        wg_dma.ins, xc0_dma.ins, sync=True, reason="x chunk 0 first"
                    )
            wts.append(wt_g)

        DR = mybir.MatmulPerfMode.DoubleRow
        for t in range(TC):
            if t == 0:
                xc = xc0
                if use_fp8:
                    x8c = x8c0
            else:
                if use_fp8:
                    x8c = x8_pool.tile([P, K8T, TW], f8)
                    nc.sync.dma_start(x8c[:], x8T[t])
                xc = x_pool.tile([P, kbt, TW], bf16)
                nc.sync.dma_start(xc[:], xT[t])

            lora = correct_all or t == MIX_T
            if lora:
                # LoRA down-projection over the full K in bf16: k < KF comes
                # from the dedicated bf16 copy of the mixed chunk.
                pu = pu_pool.tile([P, TW], mybir.dt.float32)
                for k in range(KT):
                    if use_fp8:
                        mov = xm_t[:, k, :] if k < K8T else xc[:, k - K8T, :]
                    else:
                        mov = xc[:, k, :]
                    nc.tensor.matmul(
                        pu[:], at[:, k, :], mov, start=(k == 0), stop=(k == KT - 1)
                    )
                um = u_pool.tile([P, TW], bf16)
                msl = mask_t[:, t, :] if correct_all else mask_t[:]
                nc.vector.tensor_tensor(
                    um[:], pu[:], msl, op=mybir.AluOpType.mult
                )

            for oi in range(OI):
                if use_fp8:
                    # fp8 DoubleRow part: k < KF, two k-tiles per matmul.
                    p8 = p8_pool.tile([P, TW], mybir.dt.float32)
                    for j in range(K8T // 2):
                        nc.tensor.matmul(
                            p8[:],
                            w8[:, 2 * j : 2 * j + 2, oi * P : (oi + 1) * P],
                            x8c[:, 2 * j : 2 * j + 2, :],
                            start=(j == 0),
                            stop=(j == K8T // 2 - 1),
                            perf_mode=DR,
                        )
                wt_g = wts[oi // WG]
                loc = (oi % WG) * P
                po = po_pool.tile([P, TW], mybir.dt.float32)
                for k in range(kbt):
                    nc.tensor.matmul(
                        po[:],
                        wt_g[:, k, loc : loc + P],
                        xc[:, k, :],
                        start=(k == 0),
                        stop=(k == kbt - 1 and not lora),
                    )
                if lora:
                    # LoRA up-projection accumulates into the bf16 PSUM bank.
                    nc.tensor.matmul(
                        po[:],
                        bf_t[:, oi * P : (oi + 1) * P],
                        um[:],
                        start=False,
                        stop=True,
                    )
                if use_fp8:
                    # Eviction: dequantized fp8 part + bias on Scalar, then
                    # the bf16 PSUM added on Vector.
                    t1 = o_pool.tile([P, TW], mybir.dt.float32, name="t1")
                    nc.scalar.activation(
                        t1[:],
                        p8[:],
                        mybir.ActivationFunctionType.Identity,
                        bias=bias_t[:, oi : oi + 1],
                        scale=DEQ,
                    )
                    ot = o_pool.tile([P, TW], mybir.dt.float32, name="ot")
                    nc.vector.tensor_tensor(
                        ot[:], t1[:], po[:], op=mybir.AluOpType.add
                    )
                else:
                    ot = o_pool.tile([P, TW], mybir.dt.float32, name="ot")
                    nc.scalar.add(ot[:], po[:], bias_t[:, oi : oi + 1])
                # Alternate output DMAs across both HWDGE rings from t>=1
                # (the scalar ring is still streaming W during chunk 0).
                oq = nc.sync if (t == 0 or oi % 2 == 0) else nc.scalar
                oq.dma_start(
                    outT[oi * P : (oi + 1) * P, t * TW : (t + 1) * TW], ot[:]
                )

    nc.compile()
    return nc


def _get_nc(correct_all):
    key = ("nc", correct_all)
    if key not in _CACHE:
        _CACHE[key] = _build(correct_all)
    return _CACHE[key]


def _install_trace_shim():
    """This image's antenv lacks axon_hooks; register the NTFF profile hook
    ourselves so run_bass_kernel_spmd(trace=True) can capture exec_time_ns."""
    import sys
    import types

    if "antenv.axon_hooks" in sys.modules:
        return
    import antenv

    mod = types.ModuleType("antenv.axon_hooks")
    state = {"hook": None}
    mod.set_axon_ntff_profile_hook = lambda h: state.__setitem__("hook", h)
    mod.get_axon_ntff_profile_hook = lambda: state["hook"]
    sys.modules["antenv.axon_hooks"] = mod
    antenv.axon_hooks = mod

    from trn_agent_boot.trn_boot import _ntff_profile_via_ctypes

    mod.set_axon_ntff_profile_hook(
        _ntff_profile_via_ctypes("/opt/axon/libaxon_pjrt.so")
    )

    # No S3 in this container; keep artifacts local.
    import concourse.bass_utils as bu

    bu.upload_artifacts = lambda tmpdir: f"local://{tmpdir}"


_ADAPTERS_COL = (np.arange(LR, dtype=np.int32) // R)[:, None]  # [LR, 1]


def _mask_for(sel, assigned):
    """mask'[c, j] = SCALE * ((c//R == sel[j]) - (c//R == assigned)), bf16."""
    m = (_ADAPTERS_COL == sel[None, :]).astype(np.float32)
    m -= (_ADAPTERS_COL == assigned).astype(np.float32)
    return (m * SCALE).astype(_BF16)


def kernel(x, W, b, A_all, B_all, lora_idx, _trace=False):
    global LAST_EXEC_TIME_NS
    from concourse.bass_utils import run_bass_kernel_spmd

    if _trace:
        try:
            _install_trace_shim()
        except Exception as e:  # degrade to untraced run
            print(f"trace shim failed ({e!r}); running untraced")
            _trace = False

    x = np.asarray(x, dtype=np.float32)
    W = np.asarray(W, dtype=np.float32)
    b = np.asarray(b, dtype=np.float32)
    A_all = np.asarray(A_all, dtype=np.float32)
    B_all = np.asarray(B_all, dtype=np.float32)
    lora_idx = np.asarray(lora_idx, dtype=np.int32)

    # Merged per-adapter weights: W'_a = W + SCALE * B_a @ A_a.
    Wm = W[None, :, :] + SCALE * np.matmul(B_all, A_all)  # [L, D_OUT, D_IN]

    aT_np = np.ascontiguousarray(
        A_all.reshape(LR, KT, P).astype(_BF16).transpose(2, 1, 0)
    )
    bF_np = np.ascontiguousarray(B_all.transpose(0, 2, 1)).reshape(LR, D_OUT).astype(
        _BF16
    )
    bias_np = np.ascontiguousarray(b.reshape(OI, P).T).astype(np.float32)

    # Token grouping: stable sort by adapter, then fill each core's pure
    # slots from one adapter and pool the remainder into the mixed chunks.
    cnt = np.bincount(lora_idx, minlength=L)
    order = np.argsort(lora_idx, kind="stable")
    cum = np.zeros(L + 1, dtype=np.int64)
    cum[1:] = np.cumsum(cnt)
    used = cum[:-1].copy()

    remaining = cnt.astype(np.int64).copy()
    assign = []
    ok = True
    for _ in range(M_CORES):
        a = int(np.argmax(remaining))
        if remaining[a] < PURE:
            ok = False
            break
        assign.append(a)
        remaining[a] -= PURE

    perm_cores = []
    masks = []
    if ok:
        correct_all = False
        pure = []
        for a in assign:
            pure.append(order[used[a] : used[a] + PURE])
            used[a] += PURE
        leftover = np.concatenate([order[used[a] : cum[a + 1]] for a in range(L)])
        assert leftover.size == M_CORES * TW
        for c in range(M_CORES):
            lo = leftover[c * TW : (c + 1) * TW]
            pc = np.concatenate(
                [pure[c][: MIX_T * TW], lo, pure[c][MIX_T * TW :]]
            )
            perm_cores.append(pc)
            masks.append(_mask_for(lora_idx[lo], assign[c]))  # [LR, TW]
    else:
        # Fallback: one adapter merged everywhere, correction on all chunks,
        # all-bf16 math.
        correct_all = True
        a0 = int(np.argmax(cnt))
        assign = [a0] * M_CORES
        for c in range(M_CORES):
            pc = np.arange(c * NS, (c + 1) * NS, dtype=np.int64)
            perm_cores.append(pc)
            mfull = _mask_for(lora_idx[pc], a0)  # [LR, NS]
            masks.append(
                np.ascontiguousarray(
                    mfull.reshape(LR, TC, TW).transpose(1, 0, 2)
                )
            )

    use_fp8 = not correct_all
    kf = KF if use_fp8 else 0
    kbt = KT - kf // P

    # Per-adapter device weight layouts.
    wT_by_adapter = {}
    w8_by_adapter = {}
    for a in set(assign):
        wT_by_adapter[a] = np.ascontiguousarray(
            Wm[a][:, kf:].astype(_BF16).reshape(WG, WGC, kbt, P).transpose(0, 3, 2, 1)
        )
        if use_fp8:
            w8_by_adapter[a] = np.ascontiguousarray(
                (Wm[a][:, :kf] * WSCALE)
                .astype(_F8)
                .reshape(D_OUT, K8T, P)
                .transpose(2, 1, 0)
            )

    in_maps = []
    for c in range(M_CORES):
        pc = perm_cores[c]
        xp = x[pc]  # [NS, D_IN] f32
        xT_c = np.ascontiguousarray(
            xp[:, kf:].astype(_BF16).reshape(TC, TW, kbt, P).transpose(0, 3, 2, 1)
        )
        im = {
            "xT": xT_c,
            "wT": wT_by_adapter[assign[c]],
            "aT": aT_np,
            "bF": bF_np,
            "bias": bias_np,
            "mT": masks[c],
        }
        if use_fp8:
            im["x8T"] = np.ascontiguousarray(
                (xp[:, :kf] * XSCALE)
                .astype(_F8)
                .reshape(TC, TW, K8T, P)
                .transpose(0, 3, 2, 1)
            )
            im["w8T"] = w8_by_adapter[assign[c]]
            im["xM"] = np.ascontiguousarray(
                xp[MIX_T * TW : (MIX_T + 1) * TW, :kf]
                .astype(_BF16)
                .reshape(TW, K8T, P)
                .transpose(2, 1, 0)
            )
        in_maps.append(im)

    nc = _get_nc(correct_all)
    res = run_bass_kernel_spmd(
        nc, in_maps, core_ids=list(range(M_CORES)), trace=_trace
    )
    LAST_EXEC_TIME_NS = res.exec_time_ns

    out = np.empty((N, D_OUT), dtype=np.float32)
    for c in range(M_CORES):
        out[perm_cores[c]] = res.results[c]["outT"].T
    return out


# revision 15
# speedup vs baseline: 1.1835x; 1.1835x over previous
"""Fused multi-LoRA linear layer on 8 TRN2 NeuronCores.

out = x @ W.T + b + scale * mask(x @ A_all^T) @ B_flat

Sharding: tokens are grouped by adapter on the host. Core i receives 3584
tokens of one assigned adapter (7 "pure" chunks) plus 512 leftover tokens of
mixed adapters (1 "mixed" chunk). The LoRA update for the assigned adapter is
merged into the weight on the host (W'_a = W + scale*B_a@A_a), so pure chunks
are a plain GEMM; only the mixed chunk runs the dense down-projection +
mask-select + up-projection, with mask' = scale*(onehot(sel) - onehot(assigned))
correcting the merged weight to each token's true adapter.

Precision split: the first KF=512 contraction dims run in fp8-e4m3 DoubleRow
(2 k-tiles per matmul, 2x PE throughput), accumulated in a second PSUM bank
and dequantized during eviction:

  t1 = p8 * 2^-8 + bias     (Scalar engine; x was pre-scaled by 2, W' by 128)
  ot = t1 + po              (Vector engine; po = bf16-part PSUM)

Numerically validated: total rel err ~1.6e-2 vs the 2e-2 gate (bf16-only
reference point is ~2e-3).

Device-side layout: the kernel computes out^T [d_out, tokens] so the bias and
fp8 dequant are per-partition scalars and nothing needs an on-chip transpose.
The host applies the inverse token permutation when gathering the output.
"""

import numpy as np
import ml_dtypes

# Problem constants (hardcoded per harness contract).
N, D_IN, D_OUT, L, R = 32768, 2048, 2048, 8, 16
SCALE = 32.0 / 16.0
M_CORES = 8
NS = N // M_CORES  # 4096 tokens per core
P = 128
KT = D_IN // P  # 16 k-tiles total
KF = 512  # contraction dims computed in fp8 DoubleRow
K8T = KF // P  # 4 fp8 k-tiles
KBT = KT - K8T  # 12 bf16 k-tiles
OI = D_OUT // P  # 16 output row-chunks of 128
TW = 512  # token tile width (moving free dim)
TC = NS // TW  # 8 token chunks per core
LR = L * R  # 128
WG = 4  # W column groups
WGC = D_OUT // WG  # 512 columns per group
MIX_T = 1  # chunk slot holding the mixed-adapter leftover tokens
PURE = (TC - 1) * TW  # 3584 single-adapter tokens per core

XSCALE = 2.0  # x pre-scale before e4m3 quantization
WSCALE = 128.0  # W' pre-scale before e4m3 quantization
DEQ = 1.0 / (XSCALE * WSCALE)

_BF16 = ml_dtypes.bfloat16
_F8 = ml_dtypes.float8_e4m3fn

_CACHE = {}

LAST_EXEC_TIME_NS = None


def _build(correct_all):
    import concourse.bass as bass  # noqa: F401
    import concourse.tile as tile
    from concourse import bacc, mybir
    from contextlib import ExitStack

    bf16 = mybir.dt.bfloat16
    f32 = mybir.dt.float32
    f8 = mybir.dt.float8e4
    use_fp8 = not correct_all

    nc = bacc.Bacc(
        "TRN2",
        target_bir_lowering=False,
        debug=False,
        num_devices=M_CORES,
    )

    # Host-prepared, partition-major layouts (see kernel()). In the fp8
    # variant the x/W tensors are split at k=KF into an fp8 part and a bf16
    # part; the fallback (correct_all) variant is all-bf16 with correction
    # masks on every chunk.
    kbt = KBT if use_fp8 else KT
    xT = nc.dram_tensor("xT", [TC, P, kbt, TW], bf16, kind="ExternalInput").ap()
    wT = nc.dram_tensor("wT", [WG, P, kbt, WGC], bf16, kind="ExternalInput").ap()
    if use_fp8:
        x8T = nc.dram_tensor("x8T", [TC, P, K8T, TW], f8, kind="ExternalInput").ap()
        w8T = nc.dram_tensor("w8T", [P, K8T, D_OUT], f8, kind="ExternalInput").ap()
        xM = nc.dram_tensor("xM", [P, K8T, TW], bf16, kind="ExternalInput").ap()
    aT = nc.dram_tensor("aT", [P, KT, LR], bf16, kind="ExternalInput").ap()
    bF = nc.dram_tensor("bF", [P, D_OUT], bf16, kind="ExternalInput").ap()
    bias = nc.dram_tensor("bias", [P, OI], f32, kind="ExternalInput").ap()
    mshape = [TC, P, TW] if correct_all else [P, TW]
    mT = nc.dram_tensor("mT", mshape, bf16, kind="ExternalInput").ap()
    outT = nc.dram_tensor("outT", [D_OUT, NS], f32, kind="ExternalOutput").ap()

    from concourse.tile_rust import add_dep_helper

    with tile.TileContext(nc) as tc, ExitStack() as ctx:
        warm_pool = ctx.enter_context(tc.tile_pool(name="warm", bufs=1))
        wt_pool = ctx.enter_context(tc.tile_pool(name="wt", bufs=WG))
        at_pool = ctx.enter_context(tc.tile_pool(name="at", bufs=1))
        bf_pool = ctx.enter_context(tc.tile_pool(name="bfp", bufs=1))
        bias_pool = ctx.enter_context(tc.tile_pool(name="bias", bufs=1))
        mask_pool = ctx.enter_context(tc.tile_pool(name="mask", bufs=1))
        x_pool = ctx.enter_context(tc.tile_pool(name="x", bufs=2))
        u_pool = ctx.enter_context(tc.tile_pool(name="u", bufs=2))
        o_bufs = 8 if use_fp8 else 4
        o_pool = ctx.enter_context(tc.tile_pool(name="o", bufs=o_bufs))
        if use_fp8:
            w8_pool = ctx.enter_context(tc.tile_pool(name="w8", bufs=1))
            x8_pool = ctx.enter_context(tc.tile_pool(name="x8", bufs=2))
            xm_pool = ctx.enter_context(tc.tile_pool(name="xm", bufs=1))
        pu_bufs = 2 if correct_all else 1
        pu_pool = ctx.enter_context(tc.tile_pool(name="pu", bufs=pu_bufs, space="PSUM"))
        po_pool = ctx.enter_context(tc.tile_pool(name="po", bufs=4, space="PSUM"))
        if use_fp8:
            p8_pool = ctx.enter_context(tc.tile_pool(name="p8", bufs=3, space="PSUM"))

        # Warm up the PE (HAM clock ramp) with throwaway matmuls while the
        # input DMAs stream in; sized to bridge until the first x chunk and
        # W group land (~19us). Rotating accumulation chains over po banks
        # keep the warm matmuls back-to-back.
        warm = warm_pool.tile([P, P], bf16)
        nc.vector.memset(warm[:], 0.0)
        NWB = 3
        pws = [
            po_pool.tile([P, TW], mybir.dt.float32, name="po") for i in range(NWB)
        ]
        NWARM = 112 if use_fp8 else 180
        for i in range(NWARM):
            nc.tensor.matmul(
                pws[i % NWB][:, :P],
                warm[:],
                warm[:],
                start=(i < NWB),
                stop=(i >= NWARM - NWB),
            )

        # Sync ring: first x chunk heads the queue (fp8 slice first - it is
        # small and unblocks the DoubleRow matmuls early).
        if use_fp8:
            x8c0 = x8_pool.tile([P, K8T, TW], f8)
            nc.sync.dma_start(x8c0[:], x8T[0])
        bias_t = bias_pool.tile([P, OI], f32)
        nc.sync.dma_start(bias_t[:], bias[:, :])
        xc0 = x_pool.tile([P, kbt, TW], bf16)
        xc0_dma = nc.sync.dma_start(xc0[:], xT[0])
        at = at_pool.tile([P, KT, LR], bf16)
        nc.sync.dma_start(at[:], aT[:, :, :])
        if use_fp8:
            xm_t = xm_pool.tile([P, K8T, TW], bf16)
            nc.sync.dma_start(xm_t[:], xM[:, :, :])
        if correct_all:
            mask_t = mask_pool.tile([P, TC, TW], bf16)
            nc.sync.dma_start(mask_t[:], mT.rearrange("t p j -> p t j"))
        else:
            mask_t = mask_pool.tile([P, TW], bf16)
            nc.sync.dma_start(mask_t[:], mT[:, :])

        # Scalar ring: fp8 W first (small - it feeds the DoubleRow matmuls
        # that bridge warmup to the bf16 stream), then W' group 0 + B_flat.
        # Remaining W' groups go via the GpSimd SWDGE ring so their issue
        # backpressure never blocks the Scalar engine's evictions; they are
        # gated behind the first x chunk so they don't starve it on HBM.
        wts = []
        for g in range(WG):
            wt_g = wt_pool.tile([P, kbt, WGC], bf16)
            if g == 0:
                if use_fp8:
                    w8 = w8_pool.tile([P, K8T, D_OUT], f8)
                    nc.scalar.dma_start(w8[:], w8T[:, :, :])
                nc.scalar.dma_start(wt_g[:], wT[g])
                bf_t = bf_pool.tile([P, D_OUT], bf16)
                nc.scalar.dma_start(bf_t[:], bF[:, :])
            else:
                wg_dma = nc.gpsimd.dma_start(wt_g[:], wT[g])
                if g == 1:
                    add_dep_helper(
                        wg_dma.ins, xc0_dma.ins, sync=True, reason="x chunk 0 first"
                    )
            wts.append(wt_g)

        DR = mybir.MatmulPerfMode.DoubleRow
        for t in range(TC):
            if t == 0:
                xc = xc0
                if use_fp8:
                    x8c = x8c0
            else:
                if use_fp8:
                    x8c = x8_pool.tile([P, K8T, TW], f8)
                    nc.sync.dma_start(x8c[:], x8T[t])
                xc = x_pool.tile([P, kbt, TW], bf16)
                nc.sync.dma_start(xc[:], xT[t])

            lora = correct_all or t == MIX_T
            if lora:
                # LoRA down-projection over the full K in bf16: k < KF comes
                # from the dedicated bf16 copy of the mixed chunk.
                pu = pu_pool.tile([P, TW], mybir.dt.float32)
                for k in range(KT):
                    if use_fp8:
                        mov = xm_t[:, k, :] if k < K8T else xc[:, k - K8T, :]
                    else:
                        mov = xc[:, k, :]
                    nc.tensor.matmul(
                        pu[:], at[:, k, :], mov, start=(k == 0), stop=(k == KT - 1)
                    )
                um = u_pool.tile([P, TW], bf16)
                msl = mask_t[:, t, :] if correct_all else mask_t[:]
                nc.vector.tensor_tensor(
                    um[:], pu[:], msl, op=mybir.AluOpType.mult
                )

            for oi in range(OI):
                if use_fp8:
                    # fp8 DoubleRow part: k < KF, two k-tiles per matmul.
                    p8 = p8_pool.tile([P, TW], mybir.dt.float32)
                    for j in range(K8T // 2):
                        nc.tensor.matmul(
                            p8[:],
                            w8[:, 2 * j : 2 * j + 2, oi * P : (oi + 1) * P],
                            x8c[:, 2 * j : 2 * j + 2, :],
                            start=(j == 0),
                            stop=(j == K8T // 2 - 1),
                            perf_mode=DR,
                        )
                wt_g = wts[oi // WG]
                loc = (oi % WG) * P
                po = po_pool.tile([P, TW], mybir.dt.float32)
                for k in range(kbt):
                    nc.tensor.matmul(
                        po[:],
                        wt_g[:, k, loc : loc + P],
                        xc[:, k, :],
                        start=(k == 0),
                        stop=(k == kbt - 1 and not lora),
                    )
                if lora:
                    # LoRA up-projection accumulates into the bf16 PSUM bank.
                    nc.tensor.matmul(
                        po[:],
                        bf_t[:, oi * P : (oi + 1) * P],
                        um[:],
                        start=False,
                        stop=True,
                    )
                if use_fp8:
                    # Eviction: dequantized fp8 part + bias on Scalar, then
                    # the bf16 PSUM added on Vector.
                    t1 = o_pool.tile([P, TW], mybir.dt.float32, name="t1")
                    nc.scalar.activation(
                        t1[:],
                        p8[:],
                        mybir.ActivationFunctionType.Identity,
                        bias=bias_t[:, oi : oi + 1],
                        scale=DEQ,
                    )
                    ot = o_pool.tile([P, TW], mybir.dt.float32, name="ot")
                    nc.vector.tensor_tensor(
                        ot[:], t1[:], po[:], op=mybir.AluOpType.add
                    )
                else:
                    ot = o_pool.tile([P, TW], mybir.dt.float32, name="ot")
                    nc.scalar.add(ot[:], po[:], bias_t[:, oi : oi + 1])
                # Alternate output DMAs across both HWDGE rings from t>=1
                # (the scalar ring is still streaming W during chunk 0).
                oq = nc.sync if (t == 0 or oi % 2 == 0) else nc.scalar
                oq.dma_start(
                    outT[oi * P : (oi + 1) * P, t * TW : (t + 1) * TW], ot[:]
                )

    nc.compile()
    return nc


def _get_nc(correct_all):
    key = ("nc", correct_all)
    if key not in _CACHE:
        _CACHE[key] = _build(correct_all)
    return _CACHE[key]


def _install_trace_shim():
    """This image's antenv lacks axon_hooks; register the NTFF profile hook
    ourselves so run_bass_kernel_spmd(trace=True) can capture exec_time_ns."""
    import sys
    import types

    if "antenv.axon_hooks" in sys.modules:
        return
    import antenv

    mod = types.ModuleType("antenv.axon_hooks")
    state = {"hook": None}
    mod.set_axon_ntff_profile_hook = lambda h: state.__setitem__("hook", h)
    mod.get_axon_ntff_profile_hook = lambda: state["hook"]
    sys.modules["antenv.axon_hooks"] = mod
    antenv.axon_hooks = mod

    from trn_agent_boot.trn_boot import _ntff_profile_via_ctypes

    mod.set_axon_ntff_profile_hook(
        _ntff_profile_via_ctypes("/opt/axon/libaxon_pjrt.so")
    )

    # No S3 in this container; keep artifacts local.
    import concourse.bass_utils as bu

    bu.upload_artifacts = lambda tmpdir: f"local://{tmpdir}"


_ADAPTERS_COL = (np.arange(LR, dtype=np.int32) // R)[:, None]  # [LR, 1]


def _mask_for(sel, assigned):
    """mask'[c, j] = SCALE * ((c//R == sel[j]) - (c//R == assigned)), bf16."""
    m = (_ADAPTERS_COL == sel[None, :]).astype(np.float32)
    m -= (_ADAPTERS_COL == assigned).astype(np.float32)
    return (m * SCALE).astype(_BF16)


def kernel(x, W, b, A_all, B_all, lora_idx, _trace=False):
    global LAST_EXEC_TIME_NS
    from concourse.bass_utils import run_bass_kernel_spmd

    if _trace:
        try:
            _install_trace_shim()
        except Exception as e:  # degrade to untraced run
            print(f"trace shim failed ({e!r}); running untraced")
            _trace = False

    x = np.asarray(x, dtype=np.float32)
    W = np.asarray(W, dtype=np.float32)
    b = np.asarray(b, dtype=np.float32)
    A_all = np.asarray(A_all, dtype=np.float32)
    B_all = np.asarray(B_all, dtype=np.float32)
    lora_idx = np.asarray(lora_idx, dtype=np.int32)

    # Merged per-adapter weights: W'_a = W + SCALE * B_a @ A_a.
    Wm = W[None, :, :] + SCALE * np.matmul(B_all, A_all)  # [L, D_OUT, D_IN]

    aT_np = np.ascontiguousarray(
        A_all.reshape(LR, KT, P).astype(_BF16).transpose(2, 1, 0)
    )
    bF_np = np.ascontiguousarray(B_all.transpose(0, 2, 1)).reshape(LR, D_OUT).astype(
        _BF16
    )
    bias_np = np.ascontiguousarray(b.reshape(OI, P).T).astype(np.float32)

    # Token grouping: stable sort by adapter, then fill each core's pure
    # slots from one adapter and pool the remainder into the mixed chunks.
    cnt = np.bincount(lora_idx, minlength=L)
    order = np.argsort(lora_idx, kind="stable")
    cum = np.zeros(L + 1, dtype=np.int64)
    cum[1:] = np.cumsum(cnt)
    used = cum[:-1].copy()

    remaining = cnt.astype(np.int64).copy()
    assign = []
    ok = True
    for _ in range(M_CORES):
        a = int(np.argmax(remaining))
        if remaining[a] < PURE:
            ok = False
            break
        assign.append(a)
        remaining[a] -= PURE

    perm_cores = []
    masks = []
    if ok:
        correct_all = False
        pure = []
        for a in assign:
            pure.append(order[used[a] : used[a] + PURE])
            used[a] += PURE
        leftover = np.concatenate([order[used[a] : cum[a + 1]] for a in range(L)])
        assert leftover.size == M_CORES * TW
        for c in range(M_CORES):
            lo = leftover[c * TW : (c + 1) * TW]
            pc = np.concatenate(
                [pure[c][: MIX_T * TW], lo, pure[c][MIX_T * TW :]]
            )
            perm_cores.append(pc)
            masks.append(_mask_for(lora_idx[lo], assign[c]))  # [LR, TW]
    else:
        # Fallback: one adapter merged everywhere, correction on all chunks,
        # all-bf16 math.
        correct_all = True
        a0 = int(np.argmax(cnt))
        assign = [a0] * M_CORES
        for c in range(M_CORES):
            pc = np.arange(c * NS, (c + 1) * NS, dtype=np.int64)
            perm_cores.append(pc)
            mfull = _mask_for(lora_idx[pc], a0)  # [LR, NS]
            masks.append(
                np.ascontiguousarray(
                    mfull.reshape(LR, TC, TW).transpose(1, 0, 2)
                )
            )

    use_fp8 = not correct_all
    kf = KF if use_fp8 else 0
    kbt = KT - kf // P

    # Per-adapter device weight layouts.
    wT_by_adapter = {}
    w8_by_adapter = {}
    for a in set(assign):
        wT_by_adapter[a] = np.ascontiguousarray(
            Wm[a][:, kf:].astype(_BF16).reshape(WG, WGC, kbt, P).transpose(0, 3, 2, 1)
        )
        if use_fp8:
            w8_by_adapter[a] = np.ascontiguousarray(
                (Wm[a][:, :kf] * WSCALE)
                .astype(_F8)
                .reshape(D_OUT, K8T, P)
                .transpose(2, 1, 0)
            )

    in_maps = []
    for c in range(M_CORES):
        pc = perm_cores[c]
        xp = x[pc]  # [NS, D_IN] f32
        xT_c = np.ascontiguousarray(
            xp[:, kf:].astype(_BF16).reshape(TC, TW, kbt, P).transpose(0, 3, 2, 1)
        )
        im = {
            "xT": xT_c,
            "wT": wT_by_adapter[assign[c]],
            "aT": aT_np,
            "bF": bF_np,
            "bias": bias_np,
            "mT": masks[c],
        }
        if use_fp8:
            im["x8T"] = np.ascontiguousarray(
                (xp[:, :kf] * XSCALE)
                .astype(_F8)
                .reshape(TC, TW, K8T, P)
                .transpose(0, 3, 2, 1)
            )
            im["w8T"] = w8_by_adapter[assign[c]]
            im["xM"] = np.ascontiguousarray(
                xp[MIX_T * TW : (MIX_T + 1) * TW, :kf]
                .astype(_BF16)
                .reshape(TW, K8T, P)
                .transpose(2, 1, 0)
            )
        in_maps.append(im)

    nc = _get_nc(correct_all)
    res = run_bass_kernel_spmd(
        nc, in_maps, core_ids=list(range(M_CORES)), trace=_trace
    )
    LAST_EXEC_TIME_NS = res.exec_time_ns

    out = np.empty((N, D_OUT), dtype=np.float32)
    for c in range(M_CORES):
        out[perm_cores[c]] = res.results[c]["outT"].T
    return out
